# revision 1
# baseline (speedup 1.0000x reference)
"""Trainium2 Bass kernel for a dense transformer block (self-attn + cross-attn + MLP).

Sharding: 8 cores = (batch b in 0..3) x (sequence half h in 0..1). Each core
computes its 512 query tokens end-to-end with no collectives: K/V projections
are recomputed locally over the full 1024 tokens. Keys are host-reordered to
[own 512; other 512], which makes the causal structure SPMD-uniform: own-half
chunks share one tril(128) diagonal mask, fully-masked columns are skipped in
scores/exp/AV, and the remote half's all-or-nothing mask folds into exp's
per-core bias (0 or -50).

Projections run in bf16 (weights) with f32 PSUM accumulation; q/k/softmax
probabilities and V are fp8e4, and the AV matmuls use fp8 DoubleRow (two key
chunks per instruction). LayerNorm gain/bias, the 1/sqrt(d) scale, and K/V
biases are folded into host-preprocessed weights; LN stats are batched with a
DVE-only Newton rsqrt so only Exp/Gelu ever touch ACT tables. A slice of the
exp work runs on DVE via Schraudolph bitcast to unload the ACT-bound
attention windows. The softmax denominator comes free from an appended
ones-column in V (psum row 64).
"""
import os
from contextlib import ExitStack
import numpy as np
import ml_dtypes

P = 128
C = 1024
T = 1024
TL = 512     # local tokens per core
H = 16
D = 64
F = 4096
NKC = C // P      # 8 feature chunks
NTC = T // P      # 8 token chunks (kv)
NLC = TL // P     # 4 local token chunks
NFC = F // P      # 32 mlp hidden chunks
EPS = 1e-5
NEG = -50.0

_COMPILED = None


def _build():
    from concourse import bacc, tile
    import concourse.mybir as mybir
    F32 = mybir.dt.float32
    BF16 = mybir.dt.bfloat16
    FP8 = mybir.dt.float8e4
    ADD = mybir.AluOpType.add
    MULT = mybir.AluOpType.mult
    AF = mybir.ActivationFunctionType

    nc = bacc.Bacc("TRN2", target_bir_lowering=False, debug=False, num_devices=8)

    def param(name, shape, dt):
        return nc.declare_dram_parameter(name, list(shape), dt, isOutput=False)

    xb16 = param("xb16", [T, C], BF16)
    ctx16 = param("ctx16", [T, C], BF16)
    w_ext = {}
    for nm, shape in [("wq_s", (C, C)), ("wk_s", (C, C)), ("wv_s", (C, C)),
                      ("wo_s", (C, C)), ("wq_c", (C, C)), ("wk_c", (C, C)),
                      ("wv_c", (C, C)), ("wo_c", (C, C)), ("w1f", (C, F)),
                      ("w2f", (F, C))]:
        dt8 = nm in ("wq_s", "wk_s", "wq_c", "wk_c", "wv_s", "wv_c")
        w_ext[nm] = param(nm, shape, FP8 if dt8 else BF16)
    cstf = param("cstf", [P, 2 * NKC + NFC + 2], F32)   # bq_s|bq_c|b1|nbias|schb
    cstb = param("cstb", [1, 3 * C], BF16)               # bo_s|bo_c|b2
    trilm = param("trilm", [P, P], BF16)
    out_ext = nc.declare_dram_parameter("out", [TL, C], BF16, isOutput=True)

    dbg = os.environ.get("KDBG", "")
    dbg_ext = None
    if dbg:
        dbg_ext = nc.declare_dram_parameter("dbg", [P, NFC, 1024], F32, isOutput=True)

    with TileCtx(nc, tile) as (tc, es):
        cst = es.enter_context(tc.tile_pool(name="cst", bufs=1))
        xlp = es.enter_context(tc.tile_pool(name="xlp", bufs=1))
        stg = es.enter_context(tc.tile_pool(name="stg", bufs=1))
        lntm = es.enter_context(tc.tile_pool(name="lntm", bufs=4))
        kvsrc = es.enter_context(tc.tile_pool(name="kvsrc", bufs=1))
        lnq = es.enter_context(tc.tile_pool(name="lnq", bufs=1))
        kfp = es.enter_context(tc.tile_pool(name="kfp", bufs=2))
        vap = es.enter_context(tc.tile_pool(name="vap", bufs=2))
        qfp = es.enter_context(tc.tile_pool(name="qfp", bufs=1))
        yfp = es.enter_context(tc.tile_pool(name="yfp", bufs=1))
        hfp = es.enter_context(tc.tile_pool(name="hfp", bufs=1))
        wp = es.enter_context(tc.tile_pool(name="wp", bufs=4))
        pp = es.enter_context(tc.tile_pool(name="pp", bufs=3))
        smp = es.enter_context(tc.tile_pool(name="smp", bufs=4))
        xmp = es.enter_context(tc.tile_pool(name="xmp", bufs=1))
        rbp = es.enter_context(tc.tile_pool(name="rbp", bufs=2))
        exq = es.enter_context(tc.tile_pool(name="exq", bufs=1))
        f8p = es.enter_context(tc.tile_pool(name="f8p", bufs=1))
        # PSUM split: psA rotates score/projection tiles, psY holds the
        # attention AV accumulators so the per-head-pair drain chain never
        # blocks the next pair's scores. 2+2 tiles of [128,1024]f32 = 16KB.
        psA = es.enter_context(tc.tile_pool(name="psA", bufs=2, space="PSUM"))
        psY = es.enter_context(tc.tile_pool(name="psY", bufs=2, space="PSUM"))

        class PsRot:
            def __init__(self, pools):
                self.pools = pools
                self.i = 0

            def tile(self, *a, **k):
                p = self.pools[self.i % len(self.pools)]
                k.setdefault("name", f"ps_{id(self) & 0xffff}_{self.i}")
                self.i += 1
                return p.tile(*a, **k)

        ps_dense = PsRot([psA])
        ps_solo = PsRot([psA])

        # constants
        ones_r = cst.tile([1, P], BF16, tag="ones_r")
        nc.gpsimd.memset(ones_r[:], 1.0)
        eps_t = cst.tile([P, 1], F32, tag="eps_t")
        nc.gpsimd.memset(eps_t[:], EPS)
        cstf_sb = cst.tile([P, 2 * NKC + NFC + 2], F32, tag="cstf_sb")
        nc.sync.dma_start(out=cstf_sb[:], in_=cstf.ap())
        cstb_sb = cst.tile([1, 3 * C], BF16, tag="cstb_sb")
        nc.sync.dma_start(out=cstb_sb[:], in_=cstb.ap())
        tril_sb = cst.tile([P, P], BF16, tag="tril_sb")
        nc.sync.dma_start(out=tril_sb[:], in_=trilm.ap())
        bq_s_sb = cstf_sb[:, 0:NKC]
        bq_c_sb = cstf_sb[:, NKC:2 * NKC]
        b1_sb = cstf_sb[:, 2 * NKC:2 * NKC + NFC]
        nbias_sb = cstf_sb[:, 2 * NKC + NFC:2 * NKC + NFC + 1]
        schb_sb = cstf_sb[:, 2 * NKC + NFC + 1:2 * NKC + NFC + 2]
        bo_s_sb = cstb_sb[0:1, 0:C]
        bo_c_sb = cstb_sb[0:1, C:2 * C]
        b2_sb = cstb_sb[0:1, 2 * C:3 * C]

        # prime the ACT Exp table set while startup DMAs are in flight
        prim = cst.tile([P, 1], F32, tag="prim")
        nc.scalar.activation(prim[:], eps_t[:], AF.Exp)

        # persistent activations (residual base = bf16(x) upcast on-chip)
        x_loc = xlp.tile([P, NLC, C], F32, tag="x_loc")

        SUB = mybir.AluOpType.subtract

        INT32 = mybir.dt.int32
        RSH = mybir.AluOpType.logical_shift_right
        SCH_A = float(2 ** 23 / np.log(2.0))
        SCH_B = 127 * 2 ** 23 - 366393

        def transpose_chunk(dst_fm, tcx, src_ap):
            """src [128 tok, 1024 feat] -> dst_fm[:, :, tcx-block] in one XBAR DMA."""
            nc.sync.dma_start_transpose(dst_fm[:, :, P * tcx:P * (tcx + 1)], src_ap)

        def ln_transpose(src_chunks, n_chunks, dst_fm, off=0):
            """Layernorm n_chunks of [128 tok, C] and XBAR-transpose each into
            dst_fm. Stats math is batched across chunks, and rstd comes from a
            DVE-only Newton rsqrt (bitcast seed), so the whole pipeline never
            touches an ACT table set (no Sqrt<->Exp/Gelu switches).
            """
            n = n_chunks
            s = smp.tile([P, 5 * n], F32, tag="lnstats", name="lnstats")
            sums, sq = s[:, 0:n], s[:, n:2 * n]
            for c in range(n):
                nc.vector.tensor_reduce(out=sums[:, c:c + 1],
                                        in_=src_chunks[:, off + c, :],
                                        axis=mybir.AxisListType.X, op=ADD)
                sc2 = xmp.tile([P, C], BF16, tag="xm")
                nc.scalar.activation(sc2[:], src_chunks[:, off + c, :],
                                     AF.Square, accum_out=sq[:, c:c + 1])
            mu, var, y = s[:, 2 * n:3 * n], s[:, 3 * n:4 * n], s[:, 4 * n:5 * n]
            nc.vector.tensor_scalar_mul(mu, sums, 1.0 / C)
            nc.vector.tensor_scalar(out=var, in0=sq, scalar1=1.0 / C,
                                    scalar2=EPS, op0=MULT, op1=ADD)  # E[x^2]+eps
            nc.vector.tensor_tensor(out=sums, in0=mu, in1=mu, op=MULT)
            nc.vector.tensor_tensor(out=var, in0=var, in1=sums, op=SUB)  # var+eps
            # rstd = rsqrt(var): bitcast seed + 2 Newton steps, all on DVE
            nc.vector.tensor_scalar(out=y.bitcast(INT32),
                                    in0=var.bitcast(INT32),
                                    scalar1=1, scalar2=None, op0=RSH)
            nc.vector.tensor_scalar(out=y.bitcast(INT32),
                                    in0=y.bitcast(INT32),
                                    scalar1=-1, scalar2=0x5f3759df,
                                    op0=MULT, op1=ADD)
            for _ in range(2):
                nc.vector.tensor_tensor(out=sums, in0=y, in1=y, op=MULT)
                nc.vector.tensor_tensor(out=sums, in0=sums, in1=var, op=MULT)
                nc.vector.tensor_scalar(out=sums, in0=sums, scalar1=-0.5,
                                        scalar2=1.5, op0=MULT, op1=ADD)
                nc.vector.tensor_tensor(out=y, in0=y, in1=sums, op=MULT)
            nc.vector.tensor_tensor(out=mu, in0=mu, in1=y, op=MULT)  # mu*rstd
            for c in range(n):
                lt = lntm.tile([P, C], BF16, tag="lntm")
                nc.vector.tensor_scalar(out=lt[:], in0=src_chunks[:, off + c, :],
                                        scalar1=y[:, c:c + 1],
                                        scalar2=mu[:, c:c + 1],
                                        op0=MULT, op1=SUB)
                transpose_chunk(dst_fm, off + c, lt[:])

        def load_w_piece(wext, nh, dt=BF16):
            """load [128, 8, 512] weight piece (cols nh*512..)."""
            wt = wp.tile([P, NKC, 512], dt, tag="wp")
            nc.sync.dma_start(
                out=wt[:],
                in_=wext.ap().rearrange("(c p) n -> p c n", p=P)[:, :,
                                                                 512 * nh:512 * (nh + 1)])
            return wt

        # ---- stage 1: layernorms + transposes ----
        # xb16 arrives host-reordered as [local 512 tokens; remote 512], so
        # ln1kv_fm's first TL columns ARE the local queries' layernorm:
        # ln1q_fm is a view, no separate q-side pass.
        ln1kv_fm = kvsrc.tile([P, NKC, T], BF16, tag="kvsrc")
        xb16_r = xb16.ap().rearrange("(c p) f -> p c f", p=P)
        xall = stg.tile([P, NTC, C], BF16, tag="stg")
        for tcx in range(NTC):
            nc.sync.dma_start(out=xall[:, tcx, :], in_=xb16_r[:, tcx, :])
        ln_transpose(xall, NLC, ln1kv_fm)
        ln_transpose(xall, NLC, ln1kv_fm, off=NLC)
        ln1q_fm = ln1kv_fm
        ln1kv8 = f8p.tile([P, NKC, T], FP8, tag="f8src")
        for tcx in range(NTC):
            nc.scalar.activation(ln1kv8[:, :, P * tcx:P * (tcx + 1)],
                                 ln1kv_fm[:, :, P * tcx:P * (tcx + 1)], AF.Copy)


        def proj_fm_piece(wext, mh, src_fm, n_tok, dst, bias_sb=None, wt=None,
                          rot=None):
            """dst[:, 4mh:4mh+4, :n_tok] (feature-major) = W-piece.T @ src_fm."""
            nth = n_tok // 512
            if wt is None:
                wt = load_w_piece(wext, mh)
            rot = rot or ps_dense
            for m4 in range(4):
                m = 4 * mh + m4
                ps = rot.tile([P, 1024], F32, tag="pw")
                for th in range(nth):
                    for kc in range(NKC):
                        nc.tensor.matmul(ps[:, 512 * th:512 * (th + 1)],
                                         wt[:, kc, P * m4:P * (m4 + 1)],
                                         src_fm[:, kc, 512 * th:512 * (th + 1)],
                                         start=(kc == 0), stop=(kc == NKC - 1))
                d = dst[:, m, 0:n_tok]
                if bias_sb is None:
                    nc.vector.tensor_copy(out=d, in_=ps[:, 0:n_tok])
                else:
                    nc.vector.tensor_scalar_add(d, ps[:, 0:n_tok],
                                                bias_sb[:, m:m + 1])

        def proj_fm(wext, src_fm, n_tok, dst, bias_sb=None):
            for mh in range(2):
                proj_fm_piece(wext, mh, src_fm, n_tok, dst, bias_sb)

        def proj_fm_piece_dr(wext, mh, src8, n_tok, dst, bias_sb=None):
            """fp8 DoubleRow projection piece: two feature chunks per matmul.
            src8 is the fp8 copy of the layernormed source."""
            nth = n_tok // 512
            wt = load_w_piece(wext, mh, dt=FP8)
            rot = ps_dense
            for m4 in range(4):
                m = 4 * mh + m4
                ps = rot.tile([P, 1024], F32, tag="pw")
                for th in range(nth):
                    for c in range(NKC // 2):
                        nc.tensor.matmul(
                            ps[:, 512 * th:512 * (th + 1)],
                            wt[:, 2 * c:2 * c + 2, P * m4:P * (m4 + 1)],
                            src8[:, 2 * c:2 * c + 2, 512 * th:512 * (th + 1)],
                            start=(c == 0), stop=(c == NKC // 2 - 1),
                            perf_mode=mybir.MatmulPerfMode.DoubleRow)
                d = dst[:, m, 0:n_tok]
                if bias_sb is None:
                    nc.vector.tensor_copy(out=d, in_=ps[:, 0:n_tok])
                else:
                    nc.vector.tensor_scalar_add(d, ps[:, 0:n_tok],
                                                bias_sb[:, m:m + 1])

        def proj_fm_dr(wext, src8, n_tok, dst, bias_sb=None):
            for mh in range(2):
                proj_fm_piece_dr(wext, mh, src8, n_tok, dst, bias_sb)

        def proj_v_piece(wext, fh, src_fm, v_aug, rot=None):
            """Token-major V projection into head slots 8fh..8fh+7.

            Every head slot: v at cols 0:64, ones at col 64 (so the softmax
            denominator lands on psum partition 64).
            """
            if fh == 0:
                nc.gpsimd.memset(v_aug[:, :, :, D:D + 1], 1.0)
            wt = load_w_piece(wext, fh)
            rot = rot or ps_dense
            if True:
                for tcx in range(0, NTC, 2):
                    ps = rot.tile([P, 1024], F32, tag="pw")
                    for half in range(2):
                        for kc in range(NKC):
                            nc.tensor.matmul(
                                ps[:, 512 * half:512 * (half + 1)],
                                src_fm[:, kc, P * (tcx + half):P * (tcx + half + 1)],
                                wt[:, kc, :],
                                start=(kc == 0), stop=(kc == NKC - 1))
                        pv = ps[:, 512 * half:512 * (half + 1)].rearrange(
                            "p (hh d) -> p hh d", d=D)
                        nc.scalar.activation(
                            v_aug[:, tcx + half, 8 * fh:8 * (fh + 1), 0:D],
                            pv[:], AF.Copy)

        def proj_v(wext, src_fm, v_aug):
            for fh in range(2):
                proj_v_piece(wext, fh, src_fm, v_aug)

        def proj_v_piece_dr(wext, fh, src8, v_aug):
            """fp8 DoubleRow V piece: two feature chunks per matmul."""
            if fh == 0:
                nc.gpsimd.memset(v_aug[:, :, :, D:D + 1], 1.0)
            wt = load_w_piece(wext, fh, dt=FP8)
            rot = ps_dense
            for tcx in range(0, NTC, 2):
                ps = rot.tile([P, 1024], F32, tag="pw")
                for half in range(2):
                    for c in range(NKC // 2):
                        nc.tensor.matmul(
                            ps[:, 512 * half:512 * (half + 1)],
                            src8[:, 2 * c:2 * c + 2,
                                 P * (tcx + half):P * (tcx + half + 1)],
                            wt[:, 2 * c:2 * c + 2, :],
                            start=(c == 0), stop=(c == NKC // 2 - 1),
                            perf_mode=mybir.MatmulPerfMode.DoubleRow)
                    pv = ps[:, 512 * half:512 * (half + 1)].rearrange(
                        "p (hh d) -> p hh d", d=D)
                    nc.scalar.activation(
                        v_aug[:, tcx + half, 8 * fh:8 * (fh + 1), 0:D],
                        pv[:], AF.Copy)

        def proj_v_dr(wext, src8, v_aug):
            for fh in range(2):
                proj_v_piece_dr(wext, fh, src8, v_aug)

        def attention(q_fm, k_fm, v_aug, causal, y_fm, extra=()):
            """causal path relies on [local; remote] token order:
            - own-half chunk kc: cols < 128*kc fully masked (skip), the
              128-wide diagonal block masked elementwise with tril(128),
              cols >= 128*(kc+1) fully visible (exp straight to fp8).
            - remote chunks: mask is a per-core constant, folded into exp's
              bias (0 keeps, -50 kills) -> no elementwise mask work at all.
            """
            for pr in range(H // 2):
                ys = psY.tile([P, 1024], F32, tag="py", name=f"py_{pr}")
                ptp = None
                for kc in range(NTC):
                    own = causal and kc < NLC
                    cse = P * kc if own else 0          # scores/exp extent
                    csp = P * (kc & ~1) if own else 0   # AV (chunk-pair) extent
                    sp = psA.tile([P, 1024], F32, tag="pw",
                                  name=f"sp_{pr}_{kc}")
                    for hh in range(2):
                        base = D * hh
                        nc.tensor.matmul(sp[:, 512 * hh + cse:512 * (hh + 1)],
                                         k_fm[base:base + D, pr, P * kc:P * (kc + 1)],
                                         q_fm[base:base + D, pr, cse:TL],
                                         start=True, stop=True)
                    if kc % 2 == 0:
                        ptp = pp.tile([P, 2, 1024], FP8, tag="pp")
                    ptv = ptp[:, kc % 2, :].rearrange("p (e j) -> p e j", e=2)
                    spv = sp[:].rearrange("p (e j) -> p e j", e=2)
                    if own:
                        vis = P * (kc + 1)
                        nc.scalar.activation(ptv[:, :, cse:TL],
                                             spv[:, :, cse:TL], AF.Exp)
                        nc.vector.tensor_tensor(
                            out=ptv[:, :, cse:vis], in0=ptv[:, :, cse:vis],
                            in1=tril_sb[:, None, :].to_broadcast([P, 2, P]),
                            op=MULT)
                        if csp < cse:
                            nc.gpsimd.memset(ptv[:, :, csp:cse], 0.0)
                    elif causal:
                        if kc in (5,):
                            ti = exq.tile([P, 1024], INT32, tag="exq",
                                          name=f"exq_{pr}_{kc}")
                            nc.vector.tensor_scalar(
                                out=ti[:], in0=sp[:], scalar1=SCH_A,
                                scalar2=schb_sb[:], op0=MULT, op1=ADD)
                            nc.vector.tensor_copy(out=ptp[:, kc % 2, :],
                                                  in_=ti[:].bitcast(F32))
                        else:
                            nc.scalar.activation(ptp[:, kc % 2, :], sp[:],
                                                 AF.Exp, bias=nbias_sb[:])
                    else:
                        if kc in (1, 5):
                            ti = exq.tile([P, 1024], INT32, tag="exq",
                                          name=f"exq_{pr}_{kc}")
                            nc.vector.tensor_scalar(
                                out=ti[:], in0=sp[:], scalar1=SCH_A,
                                scalar2=float(SCH_B), op0=MULT, op1=ADD)
                            nc.vector.tensor_copy(out=ptp[:, kc % 2, :],
                                                  in_=ti[:].bitcast(F32))
                        else:
                            nc.scalar.activation(ptp[:, kc % 2, :], sp[:],
                                                 AF.Exp)
                    if kc % 2 == 1:
                        for hh in range(2):
                            nc.tensor.matmul(
                                ys[0:D + 1, 512 * hh + csp:512 * (hh + 1)],
                                v_aug[:, kc - 1:kc + 1, 2 * pr + hh, :],
                                ptp[:, :, 512 * hh + csp:512 * (hh + 1)],
                                start=(kc == 1), stop=(kc == NTC - 1),
                                perf_mode=mybir.MatmulPerfMode.DoubleRow)
                if pr < len(extra):
                    extra[pr]()
                for hh in range(2):
                    ysl = ys[:, 512 * hh:512 * (hh + 1)]
                    rs = rbp.tile([P, 512], BF16, tag="rs")
                    with nc.allow_low_precision(reason="softmax denom recip"):
                        nc.vector.reciprocal(rs[D:D + 1, :], ysl[D:D + 1, :])
                    rs2 = rbp.tile([1, 512], BF16, tag="rs2")
                    nc.gpsimd.dma_start(out=rs2[0:1, :], in_=rs[D:D + 1, :])
                    rb = rbp.tile([P, 512], BF16, tag="rb")
                    nc.gpsimd.partition_broadcast(rb[:], rs2[0:1, :])
                    if hh == 0:
                        nc.vector.tensor_tensor(out=y_fm[0:D, pr, :],
                                                in0=ysl[0:D, :], in1=rb[0:D, :],
                                                op=MULT)
                    else:
                        yt = rbp.tile([P, 512], BF16, tag="yt")
                        nc.vector.tensor_tensor(out=yt[0:D, :], in0=ysl[0:D, :],
                                                in1=rb[0:D, :], op=MULT)
                        nc.sync.dma_start(out=y_fm[D:2 * D, pr, :],
                                          in_=yt[0:D, :])

        def out_proj(wext, y_fm, bias_row, dst_x):
            w0 = load_w_piece(wext, 0)
            w1 = load_w_piece(wext, 1)
            for tq in range(NLC):
                po = psY.tile([P, 1024], F32, tag="py", name=f"po_{tq}")
                for nh, wt in ((0, w0), (1, w1)):
                    ph = po[:, 512 * nh:512 * (nh + 1)]
                    for fc in range(NKC):
                        nc.tensor.matmul(ph,
                                         y_fm[:, fc, P * tq:P * (tq + 1)],
                                         wt[:, fc, :],
                                         start=(fc == 0), stop=False)
                    nc.tensor.matmul(ph, ones_r[0:1, :],
                                     bias_row[0:1, 512 * nh:512 * (nh + 1)],
                                     start=False, stop=True)
                nc.vector.tensor_tensor(out=dst_x[:, tq, :], in0=po[:],
                                        in1=dst_x[:, tq, :], op=ADD)

        # ---- stage 2: self attention (CA ctx/K/V prep interleaved into the
        # ACT-bound attention window: one step after each head-pair) ----
        k_fm = kfp.tile([P, NKC, T], FP8, tag="kfm")
        proj_fm_dr(w_ext["wk_s"], ln1kv8, T, k_fm)
        v_aug = vap.tile([P, NTC, H, D + 1], FP8, tag="vaug")
        proj_v_dr(w_ext["wv_s"], ln1kv8, v_aug)
        q_fm = qfp.tile([P, NKC, TL], FP8, tag="qfm")
        proj_fm_dr(w_ext["wq_s"], ln1kv8, TL, q_fm, bias_sb=bq_s_sb)
        for tcx in range(NLC):
            nc.vector.tensor_copy(out=x_loc[:, tcx, :], in_=xall[:, tcx, :])

        ctx_fm = kvsrc.tile([P, NKC, T], BF16, tag="kvsrc")
        ctx_r = ctx16.ap().rearrange("(c p) f -> p c f", p=P)
        k_fm2 = kfp.tile([P, NKC, T], FP8, tag="kfm")
        v_aug2 = vap.tile([P, NTC, H, D + 1], FP8, tag="vaug")

        ctx8 = f8p.tile([P, NKC, T], FP8, tag="f8src")

        def ctx_step(half):
            def f():
                ctx_half = stg.tile([P, NLC, C], BF16, tag="stg")
                nc.sync.dma_start(out=ctx_half[:],
                                  in_=ctx_r[:, NLC * half:NLC * (half + 1), :])
                for tcx in range(NLC):
                    tc2 = NLC * half + tcx
                    transpose_chunk(ctx_fm, tc2, ctx_half[:, tcx, :])
                    nc.vector.tensor_copy(
                        out=ctx8[:, :, P * tc2:P * (tc2 + 1)],
                        in_=ctx_fm[:, :, P * tc2:P * (tc2 + 1)])
            return f

        ca_steps = [ctx_step(0), ctx_step(1)]
        y_fm = yfp.tile([P, NKC, TL], BF16, tag="yfm")
        attention(q_fm, k_fm, v_aug, True, y_fm, extra=ca_steps)
        out_proj(w_ext["wo_s"], y_fm, bo_s_sb, x_loc)
        proj_fm_dr(w_ext["wk_c"], ctx8, T, k_fm2)
        proj_v_dr(w_ext["wv_c"], ctx8, v_aug2)

        # ---- stage 3: cross attention ----
        ln2q_fm = lnq.tile([P, NKC, TL], BF16, tag="lnq")
        ln_transpose(x_loc, NLC, ln2q_fm)
        ln2q8 = f8p.tile([P, NKC, TL], FP8, tag="f8src")
        for tcx in range(NLC):
            nc.vector.tensor_copy(out=ln2q8[:, :, P * tcx:P * (tcx + 1)],
                                  in_=ln2q_fm[:, :, P * tcx:P * (tcx + 1)])
        q_fm2 = qfp.tile([P, NKC, TL], FP8, tag="qfm")
        proj_fm_dr(w_ext["wq_c"], ln2q8, TL, q_fm2, bias_sb=bq_c_sb)
        y_fm2 = yfp.tile([P, NKC, TL], BF16, tag="yfm")
        attention(q_fm2, k_fm2, v_aug2, False, y_fm2)
        out_proj(w_ext["wo_c"], y_fm2, bo_c_sb, x_loc)

        # ---- stage 4: mlp ----
        ln2b_fm = lnq.tile([P, NKC, TL], BF16, tag="lnq")
        ln_transpose(x_loc, NLC, ln2b_fm)
        h_fm = hfp.tile([P, NFC, TL], BF16, tag="hfm")
        for piece in range(8):
            wt = load_w_piece(w_ext["w1f"], piece)
            for m4 in range(4):
                mc = 4 * piece + m4
                ps = ps_dense.tile([P, 1024], F32, tag="pw")
                if piece == 0:
                    for th in range(2):
                        for kc in range(NKC):
                            nc.tensor.matmul(
                                ps[:, 256 * th:256 * (th + 1)],
                                wt[:, kc, P * m4:P * (m4 + 1)],
                                ln2b_fm[:, kc, 256 * th:256 * (th + 1)],
                                start=(kc == 0), stop=(kc == NKC - 1))
                else:
                    for kc in range(NKC):
                        nc.tensor.matmul(ps[:, 0:512],
                                         wt[:, kc, P * m4:P * (m4 + 1)],
                                         ln2b_fm[:, kc, :],
                                         start=(kc == 0), stop=(kc == NKC - 1))
                nc.scalar.activation(h_fm[:, mc, :], ps[:, 0:512], AF.Gelu,
                                     bias=b1_sb[:, mc:mc + 1])
        w2r = w_ext["w2f"].ap().rearrange("(c p) n -> p c n", p=P)
        pos = [psA.tile([P, 1024], F32, tag="pw", name="po_mlp_0"),
               psA.tile([P, 1024], F32, tag="pw", name="po_mlp_1"),
               psY.tile([P, 1024], F32, tag="py", name="po_mlp_2"),
               psY.tile([P, 1024], F32, tag="py", name="po_mlp_3")]
        for kg in range(4):
            for nh in range(2):
                wt = wp.tile([P, NKC, 512], BF16, tag="wp")
                nc.sync.dma_start(out=wt[:],
                                  in_=w2r[:, NKC * kg:NKC * (kg + 1),
                                          512 * nh:512 * (nh + 1)])
                for tq in range(NLC):
                    for kc in range(NKC):
                        hc = NKC * kg + kc
                        nc.tensor.matmul(pos[tq][:, 512 * nh:512 * (nh + 1)],
                                         h_fm[:, hc, P * tq:P * (tq + 1)],
                                         wt[:, kc, :],
                                         start=(kg == 0 and kc == 0), stop=False)
        out_r = out_ext.ap().rearrange("(c p) f -> p c f", p=P)
        for tq in range(NLC):
            for nh in range(2):
                nc.tensor.matmul(pos[tq][:, 512 * nh:512 * (nh + 1)], ones_r[0:1, :],
                                 b2_sb[0:1, 512 * nh:512 * (nh + 1)],
                                 start=False, stop=True)
            ob = lntm.tile([P, C], BF16, tag="lntm")
            nc.vector.tensor_tensor(out=ob[:], in0=pos[tq][:],
                                    in1=x_loc[:, tq, :], op=ADD)
            nc.sync.dma_start(out=out_r[:, tq, :], in_=ob[:])

        # ---- output (streamed per chunk above) ----

        if dbg_ext is not None:
            tap = {"ln1kv": ln1kv_fm, "kfm": k_fm, "qfm": q_fm, "yfm": y_fm,
                   "ln2q": ln2q_fm, "ctxfm": ctx_fm, "kfm2": k_fm2, "qfm2": q_fm2,
                   "yfm2": y_fm2, "ln2b": ln2b_fm, "hfm": h_fm}[dbg]
            sh = tap.shape
            nc.gpsimd.dma_start(out=dbg_ext.ap()[:, 0:sh[1], 0:sh[2]], in_=tap[:])

    nc.compile()
    return nc


class TileCtx:
    """TileContext plus an ExitStack for pools that closes before the context."""

    def __init__(self, nc, tile_mod):
        self._tc = tile_mod.TileContext(nc)
        self._es = ExitStack()

    def __enter__(self):
        tc = self._tc.__enter__()
        self._es.__enter__()
        return tc, self._es

    def __exit__(self, *exc):
        self._es.__exit__(*exc)
        return self._tc.__exit__(*exc)


def _get_compiled():
    global _COMPILED
    if _COMPILED is None:
        _COMPILED = _build()
    return _COMPILED


def _prep_inputs(x, context, ln1_g, ln1_b, ln2_g, ln2_b,
                 sa_wq, sa_bq, sa_wk, sa_bk, sa_wv, sa_bv, sa_wo, sa_bo,
                 ca_wq, ca_bq, ca_wk, ca_bk, ca_wv, ca_bv, ca_wo, ca_bo,
                 mlp_w1, mlp_b1, mlp_w2, mlp_b2):
    bf = ml_dtypes.bfloat16
    f8 = ml_dtypes.float8_e4m3
    f32 = np.float32

    def fold(g, w, scale=1.0):
        return ((g[:, None] * w) * scale).astype(bf)

    shared = {
        "wq_s": fold(ln1_g, sa_wq, 0.125).astype(np.float32).astype(f8),
        "wk_s": fold(ln1_g, sa_wk).astype(np.float32).astype(f8),
        "wv_s": fold(ln1_g, sa_wv).astype(np.float32).astype(f8),
        "wo_s": sa_wo.astype(bf),
        "wq_c": fold(ln2_g, ca_wq, 0.125).astype(np.float32).astype(f8),
        "wk_c": ca_wk.astype(np.float32).astype(f8),
        "wv_c": ca_wv.astype(np.float32).astype(f8),
        "wo_c": ca_wo.astype(bf),
        "w1f": fold(ln2_g, mlp_w1),
        "w2f": mlp_w2.astype(bf),
    }
    bq_s = ((ln1_b @ sa_wq + sa_bq) * 0.125).astype(f32).reshape(NKC, P).T
    bq_c = ((ln2_b @ ca_wq + ca_bq) * 0.125).astype(f32).reshape(NKC, P).T
    b1c = (ln2_b @ mlp_w1 + mlp_b1).astype(f32).reshape(NFC, P).T
    cstf_base = np.concatenate([bq_s, bq_c, b1c], axis=1).astype(f32)
    shared["cstb"] = np.concatenate(
        [(sa_bo + (ln1_b @ sa_wv + sa_bv) @ sa_wo).reshape(1, C),
         (ca_bo + ca_bv @ ca_wo).reshape(1, C),
         np.asarray(mlp_b2).reshape(1, C)], axis=1).astype(bf)
    # Keys arrive on-chip in [own 512 tokens; other 512] order. Own-half
    # diagonal blocks share one tril(128) mask; the remote half is all-masked
    # (h=0, exp bias -50) or all-visible (h=1, bias 0).
    i = np.arange(P)[:, None]
    j = np.arange(P)[None, :]
    shared["trilm"] = np.where(i <= j, 1.0, 0.0).astype(bf)
    in_maps = []
    for core in range(8):
        b, h = core // 2, core % 2
        m = dict(shared)
        m["xb16"] = np.concatenate(
            [x[b, TL * h:TL * (h + 1)], x[b, TL * (1 - h):TL * (2 - h)]]
        ).astype(bf)
        m["ctx16"] = context[b].astype(bf)
        nb = np.full((P, 1), 0.0 if h == 1 else -50.0, f32)
        schb = (127 * 2 ** 23 - 366393) + nb * (2 ** 23 / np.log(2.0))
        m["cstf"] = np.concatenate([cstf_base, nb, schb.astype(f32)], axis=1)
        in_maps.append(m)
    return in_maps


def kernel(**inputs):
    from concourse.bass_utils import run_bass_kernel_spmd
    nc = _get_compiled()
    inputs = {k: np.asarray(v) for k, v in inputs.items()}
    in_maps = _prep_inputs(**inputs)
    res = run_bass_kernel_spmd(nc, in_maps, core_ids=list(range(8)))
    out = np.empty((4, T, C), np.float32)
    for core in range(8):
        b, h = core // 2, core % 2
        out[b, TL * h:TL * (h + 1)] = res.results[core]["out"].astype(np.float32)
    return out

